# revision 1
# baseline (speedup 1.0000x reference)
"""Multi-head attention (B=4, S=2048, D=1024, H=16) on 8 TRN2 NeuronCores.

Sharding: core c <- batch c//2, heads 8*(c%2) .. 8*(c%2)+8 (Megatron-style:
Wq/Wk/Wv column-parallel, Wo row-parallel). No collectives: the two partial
outputs per batch are summed on the host (plus the bo bias).

Per-core kernel strategy:
  - q^T, k^T computed directly in [head_dim, seq] layout (out = W^T.T @ X^T),
    v computed in natural [seq, head_dim] layout with a ones column appended.
  - Scores computed transposed: ST[s_k, s_q] = k . q, so softmax exp is pure
    elementwise (no max subtraction needed: scores ~ N(0,1) after 1/8 scale,
    fp32 exp cannot overflow) and no on-chip transposes are needed anywhere.
  - ctx^T[c, s_q] accumulated as v_aug^T @ exp(ST); the ones column yields the
    softmax denominator l[s_q] as psum row 64 for free.
  - Normalization by 1/l folded in before the output projection.
  - All matmuls run as float32r (full PE rate at N>=256, ~1.5e-4 rel err).
"""
import sys

sys.path.insert(0, "/opt/trn_rl_repo")
import numpy as np

import concourse.bass as bass
import concourse.bacc as bacc
import concourse.mybir as mybir
import concourse.tile as tile
from concourse.bass_utils import run_bass_kernel_spmd

f32 = mybir.dt.float32
f32r = mybir.dt.float32r
EXP = mybir.ActivationFunctionType.Exp

S = 2048          # sequence length
D = 1024          # model dim
HC = 8            # heads per core
DK = 64           # head dim
JC = HC * DK      # per-core projection width (512)
SCALE = 0.125     # 1/sqrt(DK)
N_CORES = 8


def _stage1(nc, tc, work, io, sb):
    """QKV projections -> qT_sb, kT_sb, v_sb (with ones column)."""
    SC = 256  # seq chunk for q/k projections
    nc.vector.memset(sb.v_sb[:, :, :, DK].bitcast(f32), 1.0)
    with (
        tc.tile_pool(name="w1", bufs=1) as wp,
        tc.tile_pool(name="x1", bufs=3) as xp,
        tc.tile_pool(name="ps1", bufs=4, space="PSUM") as pp,
    ):
        wq_sb = wp.tile([128, 8, JC], f32r, tag="wq")
        wk_sb = wp.tile([128, 8, JC], f32r, tag="wk")
        wv_sb = wp.tile([128, 8, JC], f32r, tag="wv")
        nc.sync.dma_start(wq_sb[:], io.wqt.rearrange("(kt p) j -> p kt j", p=128))
        nc.sync.dma_start(wk_sb[:], io.wkt.rearrange("(kt p) j -> p kt j", p=128))
        nc.sync.dma_start(wv_sb[:], io.wvt.rearrange("(kt p) j -> p kt j", p=128))
        for x_dram, w_sb, o_sb, b_sb in (
            (io.qt, wq_sb, sb.qT_sb, sb.bq_sb),
            (io.kt, wk_sb, sb.kT_sb, sb.bk_sb),
        ):
            for sc in range(S // SC):
                xq = xp.tile([128, 8, SC], f32r, tag="x")
                nc.sync.dma_start(
                    xq[:],
                    x_dram[:, sc * SC:(sc + 1) * SC].rearrange(
                        "(kt p) s -> p kt s", p=128
                    ),
                )
                for jt in range(4):
                    ps = pp.tile([128, SC], f32, tag="proj")
                    for ktile in range(8):
                        nc.tensor.matmul(
                            ps[:],
                            w_sb[:, ktile, jt * 128:(jt + 1) * 128],
                            xq[:, ktile, :],
                            start=(ktile == 0),
                            stop=(ktile == 7),
                        )
                    nc.vector.tensor_scalar_add(
                        o_sb[:, jt, sc * SC:(sc + 1) * SC],
                        ps[:],
                        b_sb[:, jt:jt + 1],
                    )
        for st in range(16):
            xv = xp.tile([128, 8, 128], f32r, tag="x")
            nc.sync.dma_start(
                xv[:],
                io.vt[:, st * 128:(st + 1) * 128].rearrange(
                    "(kt p) s -> p kt s", p=128
                ),
            )
            ps = pp.tile([128, JC], f32, tag="proj")
            for ktile in range(8):
                nc.tensor.matmul(
                    ps[:],
                    xv[:, ktile, :],
                    wv_sb[:, ktile, :],
                    start=(ktile == 0),
                    stop=(ktile == 7),
                )
            nc.vector.tensor_add(
                sb.v_sb[:, st, :, 0:DK],
                ps[:].rearrange("p (h c) -> p h c", h=HC),
                sb.bvb_sb[:].rearrange("p (h c) -> p h c", h=HC),
            )


def _stage2(nc, tc, work, io, sb):
    """Attention: scores^T -> exp -> ctx^T (+denominator) -> normalize.

    Head-serial, s_q blocked by 1024. Consecutive matmuls share the
    stationary operand (kT slice for scores, v slice for ctx) so LDWEIGHTS
    is amortized 1:4, and each exp covers [128, 1024] to halve ACT
    instruction count. Scores/exp run one k-iteration ahead of ctx.
    """
    with (
        tc.tile_pool(name="ps2st", bufs=3, space="PSUM") as pp_st,
        tc.tile_pool(name="ps2ctx", bufs=1, space="PSUM") as pp_ctx,
        tc.tile_pool(name="att", bufs=4) as att,
        tc.tile_pool(name="att2", bufs=2) as att2,
    ):
        ctxs = {}

        def emit_ctx(h, sqb, k, pt):
            c0, c1 = ctxs[(h, sqb)]
            vt = sb.v_sb[:, k, h, :]
            nc.tensor.matmul(c0[:], vt, pt[:, 0:512], start=(k == 0), stop=(k == 15))
            nc.tensor.matmul(c1[:], vt, pt[:, 512:1024], start=(k == 0), stop=(k == 15))
            if k == 15:
                jt = h // 2
                pbase = 64 * (h % 2)
                for ci, ctx in enumerate((c0, c1)):
                    sq = sqb * 2 + ci
                    r = att2.tile([1, 512], f32, tag=f"r{ci}", name=f"r_{h}_{sq}")
                    nc.vector.reciprocal(r[:], ctx[DK:DK + 1, :])
                    rb = att2.tile([64, 512], f32, tag=f"rb{ci}", name=f"rb_{h}_{sq}")
                    nc.gpsimd.partition_broadcast(rb[:], r[:])
                    nc.vector.tensor_mul(
                        sb.ctxn_sb[pbase:pbase + 64, jt, sq * 512:(sq + 1) * 512],
                        ctx[0:DK, :], rb[:],
                    )
                del ctxs[(h, sqb)]

        pend = None
        for h in range(8):
            jt = h // 2
            pbase = 64 * (h % 2)
            for sqb in range(2):      # s_q blocks of 1024
                ctxs[(h, sqb)] = (
                    pp_ctx.tile([DK + 1, 512], f32, tag="ctx0", name=f"ctx0_{h}_{sqb}"),
                    pp_ctx.tile([DK + 1, 512], f32, tag="ctx1", name=f"ctx1_{h}_{sqb}"),
                )
                for k in range(16):   # s_k tiles of 128
                    st = pp_st.tile([128, 1024], f32, tag="st")
                    lhs = sb.kT_sb[pbase:pbase + 64, jt, k * 128:(k + 1) * 128]
                    nc.tensor.matmul(
                        st[:, 0:512], lhs,
                        sb.qT_sb[pbase:pbase + 64, jt,
                                 sqb * 1024:sqb * 1024 + 512],
                        start=True, stop=True,
                    )
                    nc.tensor.matmul(
                        st[:, 512:1024], lhs,
                        sb.qT_sb[pbase:pbase + 64, jt,
                                 sqb * 1024 + 512:sqb * 1024 + 1024],
                        start=True, stop=True,
                    )
                    pt = att.tile([128, 1024], f32r, tag="pt")
                    nc.scalar.activation(pt[:], st[:], EXP, scale=SCALE)
                    if pend is not None:
                        emit_ctx(*pend)
                    pend = (h, sqb, k, pt)
        emit_ctx(*pend)


def _stage3(nc, tc, work, io, sb):
    """Output projection: out[s, :] = ctxn^T.T @ WoT."""
    with tc.tile_pool(name="ps3", bufs=2, space="PSUM") as pp3:
        for sq2 in range(16):
            for n in range(2):
                ps = pp3.tile([128, 512], f32, tag="o")
                for p in range(4):
                    nc.tensor.matmul(
                        ps[:],
                        sb.ctxn_sb[:, p, sq2 * 128:(sq2 + 1) * 128],
                        sb.wot_sb[:, p, n * 512:(n + 1) * 512],
                        start=(p == 0), stop=(p == 3),
                    )
                ob = work.tile([128, 512], f32, tag="ob")
                nc.vector.tensor_copy(ob[:], ps[:])
                nc.sync.dma_start(
                    io.out[sq2 * 128:(sq2 + 1) * 128, n * 512:(n + 1) * 512],
                    ob[:],
                )


class _NS:
    pass


def build_nc(repeats=1, stages=(1, 2, 3)):
    nc = bacc.Bacc(None, target_bir_lowering=False, debug=False)

    io = _NS()
    io.qt = nc.dram_tensor("qt", [D, S], f32r, kind="ExternalInput")
    io.kt = nc.dram_tensor("kt", [D, S], f32r, kind="ExternalInput")
    io.vt = nc.dram_tensor("vt", [D, S], f32r, kind="ExternalInput")
    io.wqt = nc.dram_tensor("wqt", [D, JC], f32r, kind="ExternalInput")
    io.wkt = nc.dram_tensor("wkt", [D, JC], f32r, kind="ExternalInput")
    io.wvt = nc.dram_tensor("wvt", [D, JC], f32r, kind="ExternalInput")
    io.wot = nc.dram_tensor("wot", [JC, D], f32r, kind="ExternalInput")
    io.bq = nc.dram_tensor("bq", [128, 4], f32, kind="ExternalInput")
    io.bk = nc.dram_tensor("bk", [128, 4], f32, kind="ExternalInput")
    io.bvb = nc.dram_tensor("bvb", [128, JC], f32, kind="ExternalInput")
    io.out = nc.dram_tensor("out", [S, D], f32, kind="ExternalOutput")

    with tile.TileContext(nc) as tc:
        for _rep in range(repeats):
            with (
                tc.tile_pool(name="big", bufs=1) as big,
                tc.tile_pool(name="work", bufs=3) as work,
            ):
                sb = _NS()
                sb.qT_sb = big.tile([128, 4, S], f32r)           # [p, jt, s]
                sb.kT_sb = big.tile([128, 4, S], f32r)
                sb.v_sb = big.tile([128, 16, HC, DK + 1], f32r)  # [p, st, h, c]
                sb.bq_sb = big.tile([128, 4], f32)
                sb.bk_sb = big.tile([128, 4], f32)
                sb.bvb_sb = big.tile([128, JC], f32)

                nc.sync.dma_start(sb.bq_sb[:], io.bq[:])
                nc.sync.dma_start(sb.bk_sb[:], io.bk[:])
                nc.sync.dma_start(sb.bvb_sb[:], io.bvb[:])

                if 1 in stages:
                    _stage1(nc, tc, work, io, sb)
                with tc.tile_pool(name="big2", bufs=1) as big2:
                    sb.ctxn_sb = big2.tile([128, 4, S], f32r)    # [p, pair, s]
                    sb.wot_sb = big2.tile([128, 4, D], f32r)
                    nc.sync.dma_start(
                        sb.wot_sb[:],
                        io.wot.rearrange("(kt p) j -> p kt j", p=128),
                    )
                    if 2 in stages:
                        _stage2(nc, tc, work, io, sb)
                    if 3 in stages:
                        _stage3(nc, tc, work, io, sb)

    nc.compile()
    return nc


_NC = None


def _get_nc():
    global _NC
    if _NC is None:
        _NC = build_nc()
    return _NC


def make_in_maps(Q, K, V, Wq, bq, Wk, bk, Wv, bv, Wo, bo):
    asf = lambda x: np.ascontiguousarray(np.asarray(x, dtype=np.float32))
    in_maps = []
    for c in range(N_CORES):
        b = c // 2
        j0 = JC * (c % 2)
        jsl = slice(j0, j0 + JC)
        in_maps.append({
            "qt": asf(np.asarray(Q)[b].T),
            "kt": asf(np.asarray(K)[b].T),
            "vt": asf(np.asarray(V)[b].T),
            "wqt": asf(np.asarray(Wq)[jsl].T),
            "wkt": asf(np.asarray(Wk)[jsl].T),
            "wvt": asf(np.asarray(Wv)[jsl].T),
            "wot": asf(np.asarray(Wo)[:, jsl].T),
            "bq": asf(np.asarray(bq)[jsl].reshape(4, 128).T),
            "bk": asf(np.asarray(bk)[jsl].reshape(4, 128).T),
            "bvb": asf(np.broadcast_to(np.asarray(bv)[jsl], (128, JC))),
        })
    return in_maps


def kernel(Q, K, V, Wq, bq, Wk, bk, Wv, bv, Wo, bo, _trace=False, _trace_kwargs=None):
    nc = _get_nc()
    in_maps = make_in_maps(Q, K, V, Wq, bq, Wk, bk, Wv, bv, Wo, bo)
    res = run_bass_kernel_spmd(
        nc, in_maps, core_ids=list(range(N_CORES)),
        trace=_trace, **(_trace_kwargs or {}),
    )
    parts = [res.results[c]["out"] for c in range(N_CORES)]
    bo_np = np.asarray(bo, dtype=np.float32)
    O = np.stack([parts[2 * b] + parts[2 * b + 1] + bo_np for b in range(4)])
    kernel.last_results = res
    return O.astype(np.float32)



# revision 6
# speedup vs baseline: 1.4322x; 1.4322x over previous
"""Multi-head attention (B=4, S=2048, D=1024, H=16) on 8 TRN2 NeuronCores.

Sharding: core c <- batch c//2, heads 8*(c%2) .. 8*(c%2)+8 (Megatron-style:
Wq/Wk/Wv column-parallel, Wo row-parallel). No collectives: the two partial
outputs per batch are summed on the host (plus the bo bias).

Per-core kernel strategy:
  - All matmul operands are fp16 (hosts converts inputs): full PE rate,
    4x cheaper LDWEIGHTS via fast-weight-load, half the DMA bytes.
  - q^T, k^T computed directly in [head_dim, seq] layout (out = W^T.T @ X^T),
    v computed in natural [seq, head_dim] layout with a ones column appended.
  - Scores computed transposed: ST[s_k, s_q] = k . q, so softmax exp is pure
    elementwise (no max subtraction needed: scores ~ N(0,1) after 1/8 scale)
    and no on-chip transposes are needed anywhere.
  - ctx^T[c, s_q] accumulated as v_aug^T @ exp(ST); the ones column yields the
    softmax denominator l[s_q] as psum row 64 for free.
  - Normalization by 1/l (reciprocal_approx_fast) before the output proj.
"""
import sys

sys.path.insert(0, "/opt/trn_rl_repo")
import numpy as np

import concourse.bass as bass
import concourse.bacc as bacc
import concourse.mybir as mybir
import concourse.tile as tile
from concourse.bass_utils import run_bass_kernel_spmd

f32 = mybir.dt.float32
f16 = mybir.dt.float16
EXP = mybir.ActivationFunctionType.Exp

S = 2048          # sequence length
D = 1024          # model dim
HC = 8            # heads per core
DK = 64           # head dim
JC = HC * DK      # per-core projection width (512)
SCALE = 0.125     # 1/sqrt(DK)
N_CORES = 8


def _stage1(nc, tc, work, io, sb):
    """QKV projections -> qT_sb, kT_sb, v_sb (with ones column)."""
    nc.vector.memset(sb.v_sb[:, :, :, DK], 1.0)
    with (
        tc.tile_pool(name="x1", bufs=2) as xp,
        tc.tile_pool(name="ps1", bufs=3, space="PSUM") as pp,
        tc.tile_pool(name="ps1v", bufs=2, space="PSUM") as ppv,
    ):
        # q/k projections: w block [128(kt),128(j)] stationary, xT moving.
        # psum [128, 1024] per (jt, chunk) accumulated over kt (LDW:MM=1:2).
        for x_dram, w_sb, o_sb, b_sb in (
            (io.qt, sb.wq_sb, sb.qT_sb, sb.bq_sb),
            (io.kt, sb.wk_sb, sb.kT_sb, sb.bk_sb),
        ):
            for sc in range(2):  # s chunks of 1024
                xq = xp.tile([128, 8, 1024], f16, tag="x")
                nc.sync.dma_start(
                    xq[:],
                    x_dram[:, sc * 1024:(sc + 1) * 1024].rearrange(
                        "(kt p) s -> p kt s", p=128
                    ),
                )
                for jt in range(4):
                    ps = pp.tile([128, 1024], f32, tag="proj")
                    for kt in range(8):
                        w = w_sb[:, kt, jt * 128:(jt + 1) * 128]
                        nc.tensor.matmul(
                            ps[:, 0:512], w, xq[:, kt, 0:512],
                            start=(kt == 0), stop=(kt == 7),
                        )
                        nc.tensor.matmul(
                            ps[:, 512:1024], w, xq[:, kt, 512:1024],
                            start=(kt == 0), stop=(kt == 7),
                        )
                    nc.vector.tensor_scalar_add(
                        o_sb[:, jt, sc * 1024:(sc + 1) * 1024],
                        ps[:],
                        b_sb[:, jt:jt + 1],
                    )
        # v projection: xT chunk stationary, wv moving; out [s, j] + bias.
        for st in range(16):
            xv = xp.tile([128, 8, 128], f16, tag="xv")
            nc.sync.dma_start(
                xv[:],
                io.vt[:, st * 128:(st + 1) * 128].rearrange(
                    "(kt p) s -> p kt s", p=128
                ),
            )
            ps = ppv.tile([128, JC], f32, tag="projv")
            for kt in range(8):
                nc.tensor.matmul(
                    ps[:],
                    xv[:, kt, :],
                    sb.wv_sb[:, kt, :],
                    start=(kt == 0), stop=(kt == 7),
                )
            nc.vector.tensor_add(
                sb.v_sb[:, st, :, 0:DK],
                ps[:].rearrange("p (h c) -> p h c", h=HC),
                sb.bvb_sb[:].rearrange("p (h c) -> p h c", h=HC),
            )


def _stage2(nc, tc, work, io, sb):
    """Attention: scores^T -> exp -> ctx^T (+denominator) -> normalize.

    Head-serial, s_q blocked by 1024. Each exp covers [128, 1024].
    Scores/exp run one k-iteration ahead of ctx.
    """
    with (
        tc.tile_pool(name="ps2st", bufs=3, space="PSUM") as pp_st,
        tc.tile_pool(name="ps2ctx", bufs=1, space="PSUM") as pp_ctx,
        tc.tile_pool(name="att", bufs=4) as att,
        tc.tile_pool(name="att2", bufs=2) as att2,
    ):
        ctxs = {}

        def emit_ctx(h, sqb, k, pt):
            c0, c1 = ctxs[(h, sqb)]
            vt = sb.v_sb[:, k, h, :]
            nc.tensor.matmul(c0[:], vt, pt[:, 0:512], start=(k == 0), stop=(k == 15))
            nc.tensor.matmul(c1[:], vt, pt[:, 512:1024], start=(k == 0), stop=(k == 15))
            if k == 15:
                jt = h // 2
                pbase = 64 * (h % 2)
                for ci, ctx in enumerate((c0, c1)):
                    sq = sqb * 2 + ci
                    r = att2.tile([1, 512], f32, tag=f"r{ci}", name=f"r_{h}_{sq}")
                    nc.vector.reciprocal(r[:], ctx[DK:DK + 1, :])
                    rb = att2.tile([64, 512], f32, tag=f"rb{ci}", name=f"rb_{h}_{sq}")
                    nc.gpsimd.partition_broadcast(rb[:], r[:])
                    nc.vector.tensor_mul(
                        sb.ctxn_sb[pbase:pbase + 64, jt, sq * 512:(sq + 1) * 512],
                        ctx[0:DK, :], rb[:],
                    )
                del ctxs[(h, sqb)]

        pend = None
        for h in range(8):
            jt = h // 2
            pbase = 64 * (h % 2)
            for sqb in range(2):      # s_q blocks of 1024
                ctxs[(h, sqb)] = (
                    pp_ctx.tile([DK + 1, 512], f32, tag="ctx0", name=f"ctx0_{h}_{sqb}"),
                    pp_ctx.tile([DK + 1, 512], f32, tag="ctx1", name=f"ctx1_{h}_{sqb}"),
                )
                for k in range(16):   # s_k tiles of 128
                    st = pp_st.tile([128, 1024], f32, tag="st")
                    lhs = sb.kT_sb[pbase:pbase + 64, jt, k * 128:(k + 1) * 128]
                    nc.tensor.matmul(
                        st[:, 0:512], lhs,
                        sb.qT_sb[pbase:pbase + 64, jt,
                                 sqb * 1024:sqb * 1024 + 512],
                        start=True, stop=True,
                    )
                    nc.tensor.matmul(
                        st[:, 512:1024], lhs,
                        sb.qT_sb[pbase:pbase + 64, jt,
                                 sqb * 1024 + 512:sqb * 1024 + 1024],
                        start=True, stop=True,
                    )
                    pt = att.tile([128, 1024], f16, tag="pt")
                    nc.scalar.activation(pt[:], st[:], EXP, scale=SCALE)
                    if pend is not None:
                        emit_ctx(*pend)
                    pend = (h, sqb, k, pt)
        emit_ctx(*pend)


def _stage3(nc, tc, work, io, sb):
    """Output projection: out[s, :] = ctxn^T.T @ WoT."""
    with tc.tile_pool(name="ps3", bufs=2, space="PSUM") as pp3:
        for sq2 in range(16):
            for n in range(2):
                ps = pp3.tile([128, 512], f32, tag="o")
                for p in range(4):
                    nc.tensor.matmul(
                        ps[:],
                        sb.ctxn_sb[:, p, sq2 * 128:(sq2 + 1) * 128],
                        sb.wot_sb[:, p, n * 512:(n + 1) * 512],
                        start=(p == 0), stop=(p == 3),
                    )
                ob = work.tile([128, 512], f32, tag="ob")
                nc.vector.tensor_copy(ob[:], ps[:])
                nc.sync.dma_start(
                    io.out[sq2 * 128:(sq2 + 1) * 128, n * 512:(n + 1) * 512],
                    ob[:],
                )


class _NS:
    pass


def build_nc(repeats=1, stages=(1, 2, 3)):
    nc = bacc.Bacc(None, target_bir_lowering=False, debug=False)

    io = _NS()
    io.qt = nc.dram_tensor("qt", [D, S], f16, kind="ExternalInput")
    io.kt = nc.dram_tensor("kt", [D, S], f16, kind="ExternalInput")
    io.vt = nc.dram_tensor("vt", [D, S], f16, kind="ExternalInput")
    io.wqt = nc.dram_tensor("wqt", [D, JC], f16, kind="ExternalInput")
    io.wkt = nc.dram_tensor("wkt", [D, JC], f16, kind="ExternalInput")
    io.wvt = nc.dram_tensor("wvt", [D, JC], f16, kind="ExternalInput")
    io.wot = nc.dram_tensor("wot", [JC, D], f16, kind="ExternalInput")
    io.bq = nc.dram_tensor("bq", [128, 4], f32, kind="ExternalInput")
    io.bk = nc.dram_tensor("bk", [128, 4], f32, kind="ExternalInput")
    io.bvb = nc.dram_tensor("bvb", [128, JC], f32, kind="ExternalInput")
    io.out = nc.dram_tensor("out", [S, D], f32, kind="ExternalOutput")

    with tile.TileContext(nc) as tc:
        for _rep in range(repeats):
            with (
                tc.tile_pool(name="big", bufs=1) as big,
                tc.tile_pool(name="work", bufs=3) as work,
            ):
                sb = _NS()
                sb.qT_sb = big.tile([128, 4, S], f16)           # [p, jt, s]
                sb.kT_sb = big.tile([128, 4, S], f16)
                sb.v_sb = big.tile([128, 16, HC, DK + 1], f16)  # [p, st, h, c]
                sb.wq_sb = big.tile([128, 8, JC], f16)
                sb.wk_sb = big.tile([128, 8, JC], f16)
                sb.wv_sb = big.tile([128, 8, JC], f16)
                sb.bq_sb = big.tile([128, 4], f32)
                sb.bk_sb = big.tile([128, 4], f32)
                sb.bvb_sb = big.tile([128, JC], f32)

                nc.sync.dma_start(sb.wq_sb[:], io.wqt.rearrange("(kt p) j -> p kt j", p=128))
                nc.sync.dma_start(sb.wk_sb[:], io.wkt.rearrange("(kt p) j -> p kt j", p=128))
                nc.sync.dma_start(sb.wv_sb[:], io.wvt.rearrange("(kt p) j -> p kt j", p=128))
                nc.sync.dma_start(sb.bq_sb[:], io.bq[:])
                nc.sync.dma_start(sb.bk_sb[:], io.bk[:])
                nc.sync.dma_start(sb.bvb_sb[:], io.bvb[:])

                if 1 in stages:
                    _stage1(nc, tc, work, io, sb)
                with tc.tile_pool(name="big2", bufs=1) as big2:
                    sb.ctxn_sb = big2.tile([128, 4, S], f16)    # [p, pair, s]
                    sb.wot_sb = big2.tile([128, 4, D], f16)
                    nc.sync.dma_start(
                        sb.wot_sb[:],
                        io.wot.rearrange("(kt p) j -> p kt j", p=128),
                    )
                    if 2 in stages:
                        _stage2(nc, tc, work, io, sb)
                    if 3 in stages:
                        _stage3(nc, tc, work, io, sb)

    nc.compile()
    return nc


_NC = None


def _get_nc():
    global _NC
    if _NC is None:
        _NC = build_nc()
    return _NC


def make_in_maps(Q, K, V, Wq, bq, Wk, bk, Wv, bv, Wo, bo):
    ash = lambda x: np.ascontiguousarray(np.asarray(x, dtype=np.float32).astype(np.float16))
    asf = lambda x: np.ascontiguousarray(np.asarray(x, dtype=np.float32))
    in_maps = []
    for c in range(N_CORES):
        b = c // 2
        j0 = JC * (c % 2)
        jsl = slice(j0, j0 + JC)
        in_maps.append({
            "qt": ash(np.asarray(Q)[b].T),
            "kt": ash(np.asarray(K)[b].T),
            "vt": ash(np.asarray(V)[b].T),
            "wqt": ash(np.asarray(Wq)[jsl].T),
            "wkt": ash(np.asarray(Wk)[jsl].T),
            "wvt": ash(np.asarray(Wv)[jsl].T),
            "wot": ash(np.asarray(Wo)[:, jsl].T),
            "bq": asf(np.asarray(bq)[jsl].reshape(4, 128).T),
            "bk": asf(np.asarray(bk)[jsl].reshape(4, 128).T),
            "bvb": asf(np.broadcast_to(np.asarray(bv)[jsl], (128, JC))),
        })
    return in_maps


def kernel(Q, K, V, Wq, bq, Wk, bk, Wv, bv, Wo, bo, _trace=False, _trace_kwargs=None):
    nc = _get_nc()
    in_maps = make_in_maps(Q, K, V, Wq, bq, Wk, bk, Wv, bv, Wo, bo)
    res = run_bass_kernel_spmd(
        nc, in_maps, core_ids=list(range(N_CORES)),
        trace=_trace, **(_trace_kwargs or {}),
    )
    parts = [res.results[c]["out"] for c in range(N_CORES)]
    bo_np = np.asarray(bo, dtype=np.float32)
    O = np.stack([parts[2 * b] + parts[2 * b + 1] + bo_np for b in range(4)])
    kernel.last_results = res
    return O.astype(np.float32)
